# revision 44
# baseline (speedup 1.0000x reference)
"""Causal self-attention (B=4, S=2048, D=768, H=12) on 8 TRN2 NeuronCores.

Sharding: batch (4) x head-group (2) = 8 cores.  Each core computes its
batch's 6 heads and a partial output projection; host sums the two group
partials plus the constant (b_v @ W_out + b_out).

v6, built from direct HW microbenchmarks (see microbench*.py):
- Every matmul runs at PE tile config (128,128)@(0,0): each tile-config
  switch (e.g. 64-contraction quadrant scores alternating with
  128-contraction AV) drains the array for ~270ns, which at several
  switches per key-chunk dominated earlier versions.  Scores use
  128-wide contraction with per-head zero-padded K tiles: k_h sits in
  its natural 64-row band, the other band is zero (written once in a
  prelude outside the timing loop), so kh.T @ q_pair selects one head
  with no quadrant placement and no extra PE cost (time = moving rows).
- All matmuls use 512-wide moving operands: narrow (256) matmuls leave
  ~100-180ns of issue/weight-load overhead exposed; at 512 wide with
  rotating stationaries the PE sustains ~107-140ns per matmul.
- One exp per (key-chunk, head) [128, <=512], contiguous from a single
  PSUM bank (ACT measured ~1.0 ns/element + ~340ns fixed; free size 512
  is near the knee).  The causal mask is a multiplicative 0/1 DVE mul
  on the exp'd diagonal square (all-SBUF bf16, 2x DVE mode).
- AV accumulates per-head into 1-bank PSUM tiles, emitted in runs of 4
  chunks to limit foreign matmuls interleaving into the open PSUM
  accumulation group (open groups interleaved with other matmuls stall
  ~1us per matmul; consecutive accumulation ~0.3us).  The late first AV
  batch (c==4) gives the previous iteration's normalization chain long
  lead time, so the av pool runs at 2 bufs and the score pool at 4 for
  deeper PE lookahead; the causal mask mul runs on the otherwise-idle
  Pool engine with ~5us of slack before its AV consumer.
- Normalization reads the av PSUM directly: reciprocal of the fused
  ones-column sums (DVE) -> PE broadcast via a [65,128] ones-row
  selector (128-col output keeps the tile config uniform; the rc tiles'
  unused rows are zeroed once so 0*garbage can't produce NaN) -> DVE
  copy + muls into apair, pipelined into the next (pair,t) iteration.
- PSUM: 3 sc + 3 av + 2 misc banks.  QKV/output projections are filler
  units paced between attention chunks so projection work fills PE gaps
  in the ACT-bound softmax stream.
Measured: ~267-275us/iter (vs 333us baseline), rel err 3.9e-3.
"""

import os

import numpy as np

import concourse.bass as bass
import concourse.tile as tile
import concourse.mybir as mybir
from concourse import bacc
from concourse._compat import with_exitstack  # noqa: F401

KBISECT = os.environ.get("KBISECT", "full")

F32 = mybir.dt.float32
BF16 = mybir.dt.bfloat16

B, S, D = 4, 2048, 768
H, DH = 12, 64
G = 2                  # head groups (tensor-parallel dimension)
HPG = H // G           # heads per group = 6
NPAIR = HPG // 2       # head pairs per group = 3
N_CORES = 8
QT = 512               # q-tile for attention
KC = 128               # key chunk
N_QT = S // QT         # 4
DC = D // 128          # 6 contraction chunks over D
WVC = HPG * (DH + 1)   # 390: V proj output cols ([V_h | 0] x 6 heads)

ACT_EL = 1.0           # measured ns per element-line on ACT
ACT_FIX = 120.0
PE_ROW = 0.27          # measured ns per moving row on PE (wide)


def declare_io(nc):
    io = {}
    io["xt"] = nc.dram_tensor("xt", [D, S], BF16, kind="ExternalInput")
    io["wqk"] = nc.dram_tensor("wqk", [D, 768], BF16, kind="ExternalInput")
    io["bqk2"] = nc.dram_tensor("bqk2", [128, 6], F32, kind="ExternalInput")
    io["wv"] = nc.dram_tensor("wv", [D, WVC], BF16, kind="ExternalInput")
    io["wo"] = nc.dram_tensor("wo", [384, 768], BF16, kind="ExternalInput")
    io["maskm"] = nc.dram_tensor("maskm", [KC, KC], BF16, kind="ExternalInput")
    io["sel"] = nc.dram_tensor("sel", [65, 128], BF16, kind="ExternalInput")
    io["onesc"] = nc.dram_tensor("onesc", [128, HPG], BF16, kind="ExternalInput")
    io["out"] = nc.dram_tensor("out", [S, D], F32, kind="ExternalOutput")
    return io


def alloc_persistent(nc, pools):
    """Tiles that live across loop iterations; zero-bands written once."""
    (consts, wqk_p, wv_p, wo_p, xt_p, qkT_p, vsb_p, pt_p, apair_p,
     bcsb_p, at_p, outsb_p, ps_sc, ps_av, ps_ms) = pools
    pers = {}
    pers["qT"] = [qkT_p.tile([128, S], BF16, tag="qT", name=f"qT{p}")
                  for p in range(NPAIR)]
    pers["kh"] = [qkT_p.tile([128, S], BF16, tag="kh", name=f"kh{h}")
                  for h in range(HPG)]
    pers["rc"] = [consts.tile([65, 2, QT], BF16, tag=f"rcs{i}", name=f"rcs{i}")
                  for i in range(2)]
    return pers


def emit_prelude(nc, pers):
    # zero the unused 64-row band of each padded K tile, and the garbage
    # rows of the reciprocal tiles (they feed a 65-partition broadcast
    # matmul whose selector rows are zero -- 0 * garbage must not be NaN)
    for h in range(HPG):
        band = pers["kh"][h][0:64, :] if h % 2 else pers["kh"][h][64:128, :]
        nc.vector.memset(band, 0.0)
    for i in range(2):
        nc.vector.memset(pers["rc"][i][0:64, :, :], 0.0)


def build_body(nc, tc, pools, pers, io):
    (consts, wqk_p, wv_p, wo_p, xt_p, qkT_p, vsb_p, pt_p, apair_p,
     bcsb_p, at_p, outsb_p, ps_sc, ps_av, ps_ms) = pools

    # ---- constants / weights into SBUF ----
    maskm_t = consts.tile([KC, KC], BF16, tag="maskm")
    nc.sync.dma_start(out=maskm_t, in_=io["maskm"][:])
    sel_t = consts.tile([65, 128], BF16, tag="sel")
    nc.sync.dma_start(out=sel_t, in_=io["sel"][:])
    bqk2_t = consts.tile([128, 6], F32, tag="bqk2")
    nc.sync.dma_start(out=bqk2_t, in_=io["bqk2"][:])
    onesc_t = consts.tile([128, HPG], BF16, tag="onesc")
    nc.sync.dma_start(out=onesc_t, in_=io["onesc"][:])

    wqk_t, xt_t = [], []
    for c in range(DC):
        w = wqk_p.tile([128, 768], BF16, tag="wqk", name=f"wqk{c}")
        nc.sync.dma_start(out=w, in_=io["wqk"][c * 128:(c + 1) * 128, :])
        wqk_t.append(w)
        x = xt_p.tile([128, S], BF16, tag="xt", name=f"xt{c}")
        nc.sync.dma_start(out=x, in_=io["xt"][c * 128:(c + 1) * 128, :])
        xt_t.append(x)
    wv_t = []
    for c in range(DC):
        w = wv_p.tile([128, WVC], BF16, tag="wv", name=f"wv{c}")
        nc.sync.dma_start(out=w, in_=io["wv"][c * 128:(c + 1) * 128, :])
        wv_t.append(w)
    wo_t = []
    for p in range(NPAIR):
        w = wo_p.tile([128, 768], BF16, tag="wo", name=f"wo{p}")
        nc.sync.dma_start(out=w, in_=io["wo"][p * 128:(p + 1) * 128, :])
        wo_t.append(w)

    qT = pers["qT"]
    kh = pers["kh"]
    rcs = pers["rc"]
    vsb = [vsb_p.tile([128, HPG, DH + 1], BF16, tag="vsb", name=f"vsb{s}")
           for s in range(S // 128)]
    apair = [apair_p.tile([128, S], BF16, tag="apair", name=f"apair{p}")
             for p in range(NPAIR)]

    # ---- emission units ----
    def emit_q_unit(p, u):
        # qT[p][:, 512u:512(u+1)] = (x Wq)^T for pair p (both heads stacked)
        pp = ps_ms.tile([128, 512], F32, tag="ms", name=f"qq{p}_{u}")
        for c in range(DC):
            nc.tensor.matmul(pp, wqk_t[c][:, 2 * p * 128:(2 * p + 1) * 128],
                             xt_t[c][:, u * 512:(u + 1) * 512],
                             start=(c == 0), stop=(c == DC - 1))
        nc.vector.tensor_scalar_add(qT[p][:, u * 512:(u + 1) * 512], pp,
                                    bqk2_t[:, 2 * p:2 * p + 1])

    def emit_k_unit(p, u):
        # k for pair p: split the two heads into their zero-padded tiles
        pp = ps_ms.tile([128, 512], F32, tag="ms", name=f"kk{p}_{u}")
        j = 2 * p + 1
        for c in range(DC):
            nc.tensor.matmul(pp, wqk_t[c][:, j * 128:(j + 1) * 128],
                             xt_t[c][:, u * 512:(u + 1) * 512],
                             start=(c == 0), stop=(c == DC - 1))
        nc.vector.tensor_scalar_add(
            kh[2 * p][0:64, u * 512:(u + 1) * 512], pp[0:64, :],
            bqk2_t[0:64, j:j + 1])
        nc.vector.tensor_scalar_add(
            kh[2 * p + 1][64:128, u * 512:(u + 1) * 512], pp[64:128, :],
            bqk2_t[64:128, j:j + 1])

    def emit_v_unit(s):
        vp = ps_ms.tile([128, WVC], F32, tag="ms", name=f"vp{s}")
        for c in range(DC):
            nc.tensor.matmul(vp, xt_t[c][:, s * 128:(s + 1) * 128], wv_t[c][:],
                             start=(c == 0), stop=(c == DC - 1))
        vv = vsb[s]
        nc.vector.tensor_copy(
            vv[:].rearrange("p h d -> p (h d)"), vp)
        nc.gpsimd.tensor_copy(vv[:, :, DH:DH + 1],
                              onesc_t[:].rearrange("p (h o) -> p h o", o=1))

    def emit_out_unit(s):
        o1 = ps_ms.tile([128, 512], F32, tag="ms", name=f"o1_{s}")
        for p in range(NPAIR):
            nc.tensor.matmul(o1, apair[p][:, s * 128:(s + 1) * 128],
                             wo_t[p][:, 0:512],
                             start=(p == 0), stop=(p == NPAIR - 1))
        o2 = ps_ms.tile([128, 256], F32, tag="ms", name=f"o2_{s}")
        for p in range(NPAIR):
            nc.tensor.matmul(o2, apair[p][:, s * 128:(s + 1) * 128],
                             wo_t[p][:, 512:768],
                             start=(p == 0), stop=(p == NPAIR - 1))
        osb = outsb_p.tile([128, D], F32, tag="outsb", name=f"osb{s}")
        nc.vector.tensor_copy(osb[:, 0:512], o1)
        nc.vector.tensor_copy(osb[:, 512:768], o2)
        nc.sync.dma_start(out=io["out"][s * 128:(s + 1) * 128, :], in_=osb)

    # ---- filler scheduling ----
    # (deadline, pe_ns, emit_fn); deadline = (pair, t) before which the unit
    # must run.  Emission order = deadline order; pacing spreads them early.
    fillers = []

    def add_qk(fn, pair, u):
        dl = (pair, min(u, N_QT - 1))
        fillers.append([dl, 6 * 512 * PE_ROW, fn])

    def add_v(s):
        dl = (0, max(0, (s - 1) // 4))
        fillers.append([dl, 6 * WVC * PE_ROW, lambda: emit_v_unit(s)])

    for s in range(4, 16):
        add_v(s)
    for u in range(1, N_QT):
        add_qk(lambda p=0, u=u: emit_q_unit(p, u), 0, u)
        add_qk(lambda p=0, u=u: emit_k_unit(p, u), 0, u)
    for pair in (1, 2):
        for u in range(N_QT):
            add_qk(lambda p=pair, u=u: emit_q_unit(p, u), pair, u)
            add_qk(lambda p=pair, u=u: emit_k_unit(p, u), pair, u)
    fillers.sort(key=lambda f: f[0])

    # total ACT ns for pacing: per (p,t), per head, one exp per key chunk
    total_act = 0.0
    for t in range(N_QT):
        n_c = 4 * t + 4
        for c in range(n_c):
            off = 0 if c < 4 * t else KC * (c - 4 * t)
            total_act += 2 * ((QT - off) * ACT_EL + ACT_FIX)
    total_act *= NPAIR

    state = dict(act_done=0.0, fill_done=0.0,
                 fill_total=sum(f[1] for f in fillers))

    def maybe_fill(force_deadline=None):
        while fillers:
            dl, pe_ns, fn = fillers[0]
            forced = force_deadline is not None and dl <= force_deadline
            paced = (state["fill_done"] < state["fill_total"]
                     * (state["act_done"] / total_act) + 2000.0)
            if not (forced or paced):
                break
            fillers.pop(0)
            fn()
            state["fill_done"] += pe_ns

    # ---- lead-in ----
    emit_q_unit(0, 0)
    emit_k_unit(0, 0)
    for s in range(4):
        emit_v_unit(s)
    if KBISECT == "upfront":
        maybe_fill(force_deadline=(98, 98))

    # ---- attention driver ----
    pending_norm = [None]

    def emit_norm_a(p, t, avh):
        # reciprocal of the fused-sums row (partition 64), per head, straight
        # from the av PSUM tiles
        rc = rcs[(p * N_QT + t) % 2]
        for j in (0, 1):
            nc.vector.reciprocal(rc[64:65, j, :], avh[j][64:65, :])
        return rc

    def emit_norm_b(p, t, avh, rc):
        # broadcast recip down the partitions via PE (ones-row selector,
        # 128-col output keeps the tile config uniform), copy to SBUF, then
        # scale (av PSUM x bcs SBUF) + store into apair.  Head 0 first: the
        # next iteration's j=1 av matmul reuses av h0's PSUM bank (3-buf
        # rotation), so freeing h0 early shortens that stall.
        bcs = bcsb_p.tile([64, 2, QT], BF16, tag="bcsb", name=f"bcs{p}_{t}")
        at = at_p.tile([64, QT], BF16, tag="at", name=f"at{p}_{t}")
        for j in (0, 1):
            bc = ps_ms.tile([128, QT], F32, tag="ms", name=f"bc{p}_{t}_{j}")
            nc.tensor.matmul(bc, sel_t, rc[0:65, j, :],
                             start=True, stop=True)
            nc.vector.tensor_copy(bcs[:, j, :], bc[0:64, :])
            if j == 0:
                nc.vector.tensor_mul(apair[p][0:64, t * QT:(t + 1) * QT],
                                     avh[0][0:64, :], bcs[:, 0, :])
            else:
                nc.vector.tensor_mul(at, avh[1][0:64, :], bcs[:, 1, :])
        nc.sync.dma_start(out=apair[p][64:128, t * QT:(t + 1) * QT], in_=at)
        if p == 2 and KBISECT != "noout":
            for s in range(4 * t, 4 * t + 4):
                fillers.append([(99, 99), 6 * 512 * PE_ROW,
                                lambda s=s: emit_out_unit(s)])
            state["fill_total"] += 4 * 6 * 512 * PE_ROW

    for p in range(NPAIR):
        for t in range(N_QT):
            maybe_fill(force_deadline=(p, t))
            n_c = 4 * t + 4
            avh = [ps_av.tile([65, QT], F32, tag="av", name=f"av{p}_{t}_{j}")
                   for j in (0, 1)]
            pt_tiles = [None] * n_c

            def emit_score_chunk(c):
                # scores for chunk c, both heads, + diagonal mask, + exp
                r = c - 4 * t
                off = 0 if r < 0 else KC * r
                ptc = []
                for j in (0, 1):
                    sc = ps_sc.tile([KC, QT], F32, tag="sc",
                                    name=f"sc{p}_{t}_{c}_{j}")
                    nc.tensor.matmul(
                        sc[:, off:QT],
                        kh[2 * p + j][:, c * KC:(c + 1) * KC],
                        qT[p][:, t * QT + off:(t + 1) * QT],
                        start=True, stop=True)
                    pt2 = pt_p.tile([KC, QT], BF16, tag="pT",
                                    name=f"pt{p}_{t}_{c}_{j}")
                    if KBISECT != "noexp":
                        nc.scalar.activation(pt2[:, off:QT], sc[:, off:QT],
                                             mybir.ActivationFunctionType.Exp)
                    if r >= 0 and KBISECT != "nomask":
                        nc.gpsimd.tensor_mul(pt2[:, off:off + KC],
                                              pt2[:, off:off + KC], maskm_t)
                    ptc.append(pt2)
                pt_tiles[c] = ptc

            def emit_av_run(c0, c1):
                # head-major: each av bank gets consecutive accumulation
                # steps (measured ~3x cheaper than alternating two open
                # banks every matmul)
                for j in (0, 1):
                    for cc in range(c0, c1):
                        r = cc - 4 * t
                        off = 0 if r < 0 else KC * r
                        nc.tensor.matmul(
                            avh[j][:, off:QT], vsb[cc][:, 2 * p + j, :],
                            pt_tiles[cc][j][:, off:QT],
                            start=(cc == 0), stop=(cc == n_c - 1),
                            skip_group_check=True)
                for cc in range(c0, c1):
                    pt_tiles[cc] = None

            skip_av = KBISECT in ("noav", "noexp")
            for c in range(n_c):
                emit_score_chunk(c)
                off = 0 if c < 4 * t else KC * (c - 4 * t)
                state["act_done"] += 2 * ((QT - off) * ACT_EL + ACT_FIX)
                if c == 0 and pending_norm[0] is not None:
                    emit_norm_b(*pending_norm[0])
                    pending_norm[0] = None
                if c >= 4 and c % 4 == 0 and not skip_av:
                    emit_av_run(c - 4, c)
                maybe_fill()
            if not skip_av:
                emit_av_run(n_c - (n_c % 4 or 4), n_c)
                if KBISECT != "avonly":
                    rc = emit_norm_a(p, t, avh)
                    pending_norm[0] = (p, t, avh, rc)

    if pending_norm[0] is not None:
        emit_norm_b(*pending_norm[0])
    pending_norm[0] = None
    maybe_fill(force_deadline=(99, 99))
    assert not fillers


def make_pools(tc, ctx):
    consts = ctx.enter_context(tc.tile_pool(name="consts", bufs=1))
    wqk_p = ctx.enter_context(tc.tile_pool(name="wqk", bufs=6))
    wv_p = ctx.enter_context(tc.tile_pool(name="wv", bufs=6))
    wo_p = ctx.enter_context(tc.tile_pool(name="wo", bufs=3))
    xt_p = ctx.enter_context(tc.tile_pool(name="xt", bufs=6))
    qkT_p = ctx.enter_context(tc.tile_pool(name="qkT", bufs=9))
    vsb_p = ctx.enter_context(tc.tile_pool(name="vsb", bufs=16))
    pt_p = ctx.enter_context(tc.tile_pool(name="pT", bufs=16))
    apair_p = ctx.enter_context(tc.tile_pool(name="apair", bufs=3))
    bcsb_p = ctx.enter_context(tc.tile_pool(name="bcsb", bufs=2))
    at_p = ctx.enter_context(tc.tile_pool(name="at", bufs=2))
    outsb_p = ctx.enter_context(tc.tile_pool(name="outsb", bufs=2))
    ps_sc = ctx.enter_context(tc.tile_pool(name="ps_sc", bufs=4, space="PSUM"))
    ps_av = ctx.enter_context(tc.tile_pool(name="ps_av", bufs=2, space="PSUM"))
    ps_ms = ctx.enter_context(tc.tile_pool(name="ps_ms", bufs=2, space="PSUM"))
    return (consts, wqk_p, wv_p, wo_p, xt_p, qkT_p, vsb_p, pt_p, apair_p,
            bcsb_p, at_p, outsb_p, ps_sc, ps_av, ps_ms)


def build_nc(n_iters=None, phases=None):
    from contextlib import ExitStack

    nc = bacc.Bacc(trn_type="TRN2", debug=False)
    nc._allow_low_precision_reason = "bf16 kept within 2e-2 tolerance"
    io = declare_io(nc)
    with tile.TileContext(nc) as tc:
        with ExitStack() as ctx:
            pools = make_pools(tc, ctx)
            pers = alloc_persistent(nc, pools)
            emit_prelude(nc, pers)
            if n_iters is None:
                build_body(nc, tc, pools, pers, io)
            else:
                with tc.For_i(0, n_iters, 1):
                    build_body(nc, tc, pools, pers, io)
    nc.compile()
    return nc, io


def host_inputs(x, W_qkv, b_qkv, W_out, b_out):
    """Per-core in_maps + the host-side unshard constant."""
    bf16 = mybir.dt.np(BF16)
    x = np.asarray(x, dtype=np.float32)
    W_qkv = np.asarray(W_qkv, dtype=np.float32)
    b_qkv = np.asarray(b_qkv, dtype=np.float32)
    W_out = np.asarray(W_out, dtype=np.float32)
    b_out = np.asarray(b_out, dtype=np.float32)

    Wq, Wk, Wv = W_qkv[:, 0:D], W_qkv[:, D:2 * D], W_qkv[:, 2 * D:3 * D]
    bq, bk, bv = b_qkv[0:D], b_qkv[D:2 * D], b_qkv[2 * D:3 * D]
    scale = 1.0 / np.sqrt(DH)

    # multiplicative causal mask for the diagonal squares: keep q >= key
    maskm = (np.arange(KC)[None, :] >= np.arange(KC)[:, None]).astype(bf16)
    sel = np.zeros((65, 128), bf16)
    sel[64, :] = 1.0
    onesc = np.ones((128, HPG), bf16)

    per_group = []
    for g in range(G):
        cols, bcols = [], []
        for p in range(NPAIR):
            h0 = g * HPG + 2 * p
            cols.append(Wq[:, h0 * DH:(h0 + 2) * DH] * scale)
            cols.append(Wk[:, h0 * DH:(h0 + 2) * DH])
            bcols.append(bq[h0 * DH:(h0 + 2) * DH] * scale)
            bcols.append(bk[h0 * DH:(h0 + 2) * DH])
        wqk_g = np.concatenate(cols, axis=1).astype(bf16)       # [768, 768]
        bqk_g = np.stack(bcols, axis=1).astype(np.float32)      # [128, 6]
        wv_g = np.zeros((D, WVC), np.float32)
        for h in range(HPG):
            hg = g * HPG + h
            wv_g[:, h * 65:h * 65 + DH] = Wv[:, hg * DH:(hg + 1) * DH]
        wo_g = W_out[g * HPG * DH:(g + 1) * HPG * DH, :].astype(bf16)
        per_group.append((wqk_g, bqk_g, wv_g.astype(bf16), wo_g))

    xt_b = [np.ascontiguousarray(x[b].T).astype(bf16) for b in range(B)]

    in_maps = []
    for core in range(N_CORES):
        b, g = core // G, core % G
        wqk_g, bqk_g, wv_g, wo_g = per_group[g]
        in_maps.append(dict(
            xt=xt_b[b], wqk=wqk_g, bqk2=bqk_g, wv=wv_g, wo=wo_g,
            maskm=maskm, sel=sel, onesc=onesc,
        ))
    cvec = (bv @ W_out + b_out).astype(np.float32)              # [768]
    return in_maps, cvec


_CACHE = {}


def kernel(x, W_qkv, b_qkv, W_out, b_out):
    from concourse.bass_utils import run_bass_kernel_spmd

    if "nc" not in _CACHE:
        _CACHE["nc"], _ = build_nc()
    nc = _CACHE["nc"]
    in_maps, cvec = host_inputs(x, W_qkv, b_qkv, W_out, b_out)
    res = run_bass_kernel_spmd(nc, in_maps, list(range(N_CORES)))
    out = np.empty((B, S, D), np.float32)
    for b in range(B):
        out[b] = res.results[2 * b]["out"] + res.results[2 * b + 1]["out"] + cvec
    return out
